# revision 80
# baseline (speedup 1.0000x reference)
"""Compositional-attention transformer block on 8 Trainium2 NeuronCores.

Sharding: core c in 0..7 handles batch b = c//2 and query-token half
h = c%2 (512 of 1024 tokens). Keys/values span the whole batch, so each
core computes LN + k/v over all 1024 tokens of its batch (duplicated
between the core pair) and q/attention/FFN for its 512 query tokens.
No collectives are needed.

Device layout is feature-major ([D, tokens]) so every linear's output
feeds the next matmul without transposes; LN statistics over the
partition (feature) axis are computed with an all-ones matmul that
leaves the per-token sums broadcast across all 128 partitions.

Key optimizations over the straightforward version:
- All weights are prefetched to SBUF during the LN1/x-load window, so
  no phase waits on a weight DMA.
- Both LayerNorms are folded into the consuming matmuls: the device
  computes xn' = x*rstd only, and the -mean*rstd correction enters each
  projection as one extra rank-1 accumulation (aug_* host vectors x the
  runtime amr row). g/b LN affines are folded host-side.
- Attention epilogue: the rule-selection softmax is applied
  UNNORMALIZED (softmax over R is scale/shift-invariant): exp(dps*rinv)
  weights multiply the raw attended values straight out of PSUM on DVE,
  a bf16 add-tree (split DVE/GPSIMD) reduces over rules, and one final
  per-query scale applies einv*rinv off the critical chain.
- attn token->feature transposes run on the DMA xbar
  (dma_start_transpose) during attention; GPSIMD handles SBUF-only
  copies/adds (it cannot touch PSUM - hardware constraint).

Math note: in the reference, the qv/Wqv/Ws[:, :32]/bs terms are
constant along the rule axis R, so they cancel inside the rule softmax
(softmax over R is shift-invariant). That whole pathway is dead code
and is not computed here.
"""

import sys

if "/opt/trn_rl_repo" not in sys.path:
    sys.path.insert(0, "/opt/trn_rl_repo")

import numpy as np

import bass_rust
import concourse.bass as bass
import concourse.mybir as mybir
from concourse.tile import TileContext
from concourse.bass_utils import run_bass_kernel_spmd

F32 = mybir.dt.float32
BF16 = mybir.dt.bfloat16

D = 1024      # model dim
S = 1024      # kv tokens per batch
Q = 512       # query tokens per core
H = 16        # heads
HD = 64       # head dim
R = 8         # rules
KC = D // 128   # 8 k-chunks of 128
QC = Q // 128   # 4 query chunks of 128
EPS = 1e-5


class _TC(TileContext):
    """TileContext whose tail drain splits its sem waits across NOPs.

    The walrus build in this environment accepts at most one sync-wait
    per CTRL instruction, so the stock single multi-wait drain fails
    codegen ("Too many sync wait commands")."""

    def _drain_and_barrier(self, tick_clock, wait_clock):
        gc = tick_clock.global_clock
        for p, t in enumerate(list(gc)):
            if t <= 0:
                continue
            sc = bass_rust.ScopedClock()
            sc.require_at_least(None, p, t)
            nop = self.nc.sync.nop(nofuse=True, hint="tail_wait")
            wait_clock.add_sem_waits(nop.ins, sc)
        self.nc.sync.drain()
        self.nc.all_engine_barrier()
        assert self.sems is not None
        popped = self.nc._tile_sem_poison_stack.pop()
        assert popped is self._sem_poison
        self.nc.clear_and_free_semaphores(list(self.sems.allocated().values()))
        self.nc.all_engine_barrier()


def _split_multi_waits(nc):
    """walrus here caps sync waits at 1 per instruction: hoist extras onto
    single-wait EventSemaphore instructions inserted just before, on the
    same engine (engine program order preserves the wait semantics)."""
    n = 0
    for fn in nc.m.functions:
        for bb in fn.blocks:
            out = []
            changed = False
            for inst in bb.instructions:
                si = inst.sync_info
                if si is not None and len(si.on_wait) > 1:
                    waits = list(si.on_wait)
                    for w in waits[:-1]:
                        ev = mybir.InstEventSemaphore(
                            name=f"I-wsp{nc.next_id()}", ins=[], outs=[],
                            sync_info=mybir.SyncInfo(on_wait=[w], on_update=[]),
                        )
                        ev.engine = inst.engine
                        out.append(ev)
                        n += 1
                    inst.sync_info = mybir.SyncInfo(
                        on_wait=[waits[-1]], on_update=list(si.on_update)
                    )
                    changed = True
                out.append(inst)
            if changed:
                bb.instructions = out
    return n


def _build_nc():
    nc = bass.Bass(target_bir_lowering=False, trn_type="TRN2")
    AF = mybir.ActivationFunctionType
    OP = mybir.AluOpType

    x_d = nc.dram_tensor("x_fm", [D, S], F32, kind="ExternalInput")
    wq_d = nc.dram_tensor("wqT", [D, D], mybir.dt.float8e4, kind="ExternalInput")
    wk_d = nc.dram_tensor("wkT", [D, D], mybir.dt.float8e4, kind="ExternalInput")
    wv_d = nc.dram_tensor("wvT", [D, R * HD], mybir.dt.float8e4, kind="ExternalInput")
    wf_d = nc.dram_tensor("wfT", [D, D], mybir.dt.float8e4, kind="ExternalInput")
    wr1_d = nc.dram_tensor("wr1T", [D, 2 * D], mybir.dt.float8e4, kind="ExternalInput")
    wr2_d = nc.dram_tensor("wr2T", [2 * D, D], mybir.dt.float8e4, kind="ExternalInput")
    bk_d = nc.dram_tensor("bk", [D], F32, kind="ExternalInput")
    bq8_d = nc.dram_tensor("bq8", [D], F32, kind="ExternalInput")
    bf_d = nc.dram_tensor("bf", [D], F32, kind="ExternalInput")
    br1_d = nc.dram_tensor("br1", [2 * D], F32, kind="ExternalInput")
    br2_d = nc.dram_tensor("br2", [D], F32, kind="ExternalInput")
    bv_d = nc.dram_tensor("bv_bc", [128, R * HD], F32, kind="ExternalInput")
    wu8_d = nc.dram_tensor("wu8", [D, 9], mybir.dt.float8e4, kind="ExternalInput")
    bu_d = nc.dram_tensor("bu_bc", [128, 9], F32, kind="ExternalInput")
    FP8 = mybir.dt.float8e4
    augk_d = nc.dram_tensor("aug_k", [2, D], FP8, kind="ExternalInput")
    augq_d = nc.dram_tensor("aug_q", [2, D], FP8, kind="ExternalInput")
    augv_d = nc.dram_tensor("aug_v", [2, R * HD], FP8, kind="ExternalInput")
    augu_d = nc.dram_tensor("aug_u", [2, 9], FP8, kind="ExternalInput")
    augr1_d = nc.dram_tensor("aug_r1", [2, 2 * D], FP8, kind="ExternalInput")
    ones_d = nc.dram_tensor("ones_c", [128, 128], F32, kind="ExternalInput")
    ident_d = nc.dram_tensor("ident_c", [128, 128], F32, kind="ExternalInput")
    consts_d = nc.dram_tensor("consts_c", [128, 2], F32, kind="ExternalInput")
    out_d = nc.dram_tensor("out_fm", [D, Q], F32, kind="ExternalOutput")

    x_pct = x_d[:, :].rearrange("(c p) t -> p c t", p=128)

    with _TC(nc) as tc:
        with tc.tile_pool(name="persist", bufs=1) as pp:
            xqb = pp.tile([128, KC, Q], F32)
            ones_sb = pp.tile([128, 128], F32)
            consts_sb = pp.tile([128, 2], F32)
            # register constants used as implicit activation biases
            nc.const_aps.aps[(F32, 0.0)] = consts_sb[:, 0:1]
            nc.const_aps.aps[(F32, EPS)] = consts_sb[:, 1:2]
            bk_sb = pp.tile([128, KC], F32)
            bq8_sb = pp.tile([128, KC], F32)
            bf_sb = pp.tile([128, KC], F32)
            br1_sb = pp.tile([128, 2 * KC], F32)
            br2_sb = pp.tile([128, KC], F32)
            bv_sb = pp.tile([128, R * HD], F32)
            bu_sb = pp.tile([128, 9], F32)

            wq_p = wq_d[:, :].rearrange("(c p) o -> p c o", p=128)
            wk_p = wk_d[:, :].rearrange("(c p) o -> p c o", p=128)
            wv_p = wv_d[:, :].rearrange("(c p) o -> p c o", p=128)
            wf_p = wf_d[:, :].rearrange("(c p) o -> p c o", p=128)
            wr1_p = wr1_d[:, :].rearrange("(c p) o -> p c o", p=128)
            wr2_p = wr2_d[:, :].rearrange("(c p) o -> p c o", p=128)
            wk_sb = pp.tile([128, KC, D], mybir.dt.float8e4)
            wq_sb = pp.tile([128, KC, D], mybir.dt.float8e4)
            wv_sb = pp.tile([128, KC, R * HD], mybir.dt.float8e4)
            wu8_sb = pp.tile([128, KC, 9], mybir.dt.float8e4)
            wf_sb = pp.tile([128, KC, D], mybir.dt.float8e4)

            augk_sb = pp.tile([1, 2, D], mybir.dt.float8e4)
            augq_sb = pp.tile([1, 2, D], mybir.dt.float8e4)
            augv_sb = pp.tile([1, 2, R * HD], mybir.dt.float8e4)
            augu_sb = pp.tile([1, 2, 9], mybir.dt.float8e4)
            augr1_sb = pp.tile([1, 2, 2 * D], mybir.dt.float8e4)
            ones_bf = pp.tile([128, 128], BF16)

            attn_sb = pp.tile([128, QC, D], BF16)   # token-major attn output
            y2_sb = pp.tile([128, KC, Q], F32)     # post-Wf residual stream

            # k/q/v live from QKV until end of attention
            akv_ctx = tc.tile_pool(name="akv", bufs=1)
            akv = akv_ctx.__enter__()
            k_sb = akv.tile([128, KC, S], BF16)
            q_sb = akv.tile([128, KC, Q], BF16)
            v_sb = akv.tile([128, KC, R * HD], mybir.dt.float8e4)
            u8_sb = akv.tile([128, KC, 9], mybir.dt.float8e4)

            with tc.tile_pool(name="mid", bufs=1) as mid:
                # tiny consts first (the stats matmuls need ones_bf), then
                # x — every compute chain waits on it — then weights
                nc.sync.dma_start(ones_sb[:], ones_d[:, :])
                nc.sync.dma_start(consts_sb[:], consts_d[:, :])
                nc.vector.tensor_copy(ones_bf[:], ones_sb[:])
                xfm = mid.tile([128, KC, S], F32)
                for kc in range(KC):
                    nc.sync.dma_start(xfm[:, kc], x_pct[:, kc])
                for half in range(2):
                    hs = slice(half * 512, (half + 1) * 512)
                    nc.sync.dma_start(wk_sb[:, :, hs], wk_p[:, :, hs])
                    nc.sync.dma_start(wq_sb[:, :, hs], wq_p[:, :, hs])
                nc.sync.dma_start(wv_sb[:], wv_p)
                nc.sync.dma_start(
                    wu8_sb[:], wu8_d[:, :].rearrange("(c p) o -> p c o", p=128))
                nc.sync.dma_start(augk_sb[:], augk_d[:, :].rearrange("(a r) o -> a r o", a=1))
                nc.sync.dma_start(augq_sb[:], augq_d[:, :].rearrange("(a r) o -> a r o", a=1))
                nc.sync.dma_start(augv_sb[:], augv_d[:, :].rearrange("(a r) o -> a r o", a=1))
                nc.sync.dma_start(augu_sb[:], augu_d[:, :].rearrange("(a r) o -> a r o", a=1))
                nc.sync.dma_start(bk_sb[:], bk_d[:].rearrange("(c p) -> p c", p=128))
                nc.sync.dma_start(bq8_sb[:], bq8_d[:].rearrange("(c p) -> p c", p=128))
                nc.sync.dma_start(bf_sb[:], bf_d[:].rearrange("(c p) -> p c", p=128))
                nc.sync.dma_start(bv_sb[:], bv_d[:, :])
                nc.sync.dma_start(bu_sb[:], bu_d[:, :])
                nc.sync.dma_start(br1_sb[:], br1_d[:].rearrange("(c p) -> p c", p=128))
                nc.sync.dma_start(br2_sb[:], br2_d[:].rearrange("(c p) -> p c", p=128))
                nc.sync.dma_start(augr1_sb[:], augr1_d[:, :].rearrange("(a r) o -> a r o", a=1))
                for half in range(2):
                    hs = slice(half * 512, (half + 1) * 512)
                    nc.sync.dma_start(wf_sb[:, :, hs], wf_p[:, :, hs])

                x_bf = mid.tile([128, KC, S], BF16)
                amr = mid.tile([1, 2, S], mybir.dt.float8e4)
                nc.vector.memset(amr[:, 1], 0.0)
                for mc in range(KC):
                    nc.gpsimd.tensor_scalar_add(
                        xqb[:, mc], xfm[:, mc, :Q], bf_sb[:, mc:mc + 1])
                xn_sb = mid.tile([128, KC, S], mybir.dt.float8e4)

                # ---------------- LN1 (feature-major) ----------------
                # xn' = x*rstd only; the -mean*rstd correction is folded
                # into every projection as one extra rank-1 matmul with the
                # amr row (see aug_* host vectors).
                with (
                    tc.tile_pool(name="ln1", bufs=1) as ln1,
                    tc.tile_pool(name="ln1sq", bufs=2) as ln1sq,
                    tc.tile_pool(name="ln1ps", bufs=1, space="PSUM") as ln1ps,
                ):
                    s_ps = [ln1ps.tile([128, 512], F32, name=f"s_ps{i}") for i in range(2)]
                    q_ps = [ln1ps.tile([128, 512], F32, name=f"q_ps{i}") for i in range(2)]
                    for kc in range(KC):
                        nc.vector.tensor_copy(x_bf[:, kc], xfm[:, kc])
                        sq_c = ln1sq.tile([128, S], BF16, tag="sq_c")
                        nc.vector.tensor_mul(sq_c[:], x_bf[:, kc], x_bf[:, kc])
                        for nh in range(2):
                            nc.tensor.matmul(
                                s_ps[nh][:], ones_bf[:],
                                x_bf[:, kc, nh * 512:(nh + 1) * 512],
                                start=(kc == 0), stop=(kc == KC - 1),
                            )
                            nc.tensor.matmul(
                                q_ps[nh][:], ones_bf[:],
                                sq_c[:, nh * 512:(nh + 1) * 512],
                                start=(kc == 0), stop=(kc == KC - 1),
                            )

                    # process per token-half so xn(half0) lands while half1's
                    # var chain still runs; D*var = q - s^2/D, with the /D
                    # folded into the Ln scale
                    rstd_sb = ln1.tile([128, S], BF16)
                    var_sb = ln1.tile([128, S], F32)
                    m2_sb = ln1.tile([128, S], F32)
                    for nh in range(2):
                        sl = slice(nh * 512, (nh + 1) * 512)
                        # m2 = s^2/D on ACT (DVE cannot read 2 PSUM inputs)
                        nc.scalar.activation(m2_sb[:, sl], s_ps[nh][:],
                                             AF.Square, scale=D ** -0.5)
                        nc.vector.scalar_tensor_tensor(
                            var_sb[:, sl], m2_sb[:, sl], -1.0,
                            q_ps[nh][:], OP.mult, OP.add,
                        )
                        # rstd = exp(-0.5*ln(var/D+eps))
                        nc.scalar.activation(var_sb[:, sl], var_sb[:, sl],
                                             AF.Ln, bias=EPS, scale=1.0 / D)
                        nc.scalar.activation(rstd_sb[:, sl], var_sb[:, sl],
                                             AF.Exp, scale=-0.5)
                        # amr = -32*mean*rstd = -(32/D)*s*rstd (fp8 row)
                        nc.vector.scalar_tensor_tensor(
                            amr[:, 0, sl], s_ps[nh][0:1, :], -32.0 / D,
                            rstd_sb[0:1, sl], OP.mult, OP.mult,
                        )
                        for kc in range(KC):
                            nc.vector.tensor_mul(xn_sb[:, kc, sl],
                                                 x_bf[:, kc, sl],
                                                 rstd_sb[:, sl])

                # ---------------- q, k, v linears ----------------
                with (
                    tc.tile_pool(name="qkvps", bufs=2, space="PSUM") as qkvps,
                    tc.tile_pool(name="upsp", bufs=2, space="PSUM") as upsp,
                ):
                    DRM = mybir.MatmulPerfMode.DoubleRow
                    for mc in range(KC):
                        osl = slice(mc * 128, (mc + 1) * 128)
                        for nh in range(2):
                            hsl = slice(nh * 512, (nh + 1) * 512)
                            kps = qkvps.tile([128, 512], F32, tag="kps")
                            for c in range(KC // 2):
                                nc.tensor.matmul(
                                    kps[:], wk_sb[:, 2 * c:2 * c + 2, osl],
                                    xn_sb[:, 2 * c:2 * c + 2, hsl],
                                    start=(c == 0), stop=False,
                                    perf_mode=DRM,
                                )
                            nc.tensor.matmul(
                                kps[:], augk_sb[:, :, osl], amr[:, :, hsl],
                                start=False, stop=True, perf_mode=DRM,
                            )
                            nc.scalar.activation(
                                k_sb[:, mc, hsl], kps[:],
                                AF.Identity, bias=bk_sb[:, mc:mc + 1],
                                scale=1.0 / 32.0,
                            )

                    for mc in range(KC):
                        osl = slice(mc * 128, (mc + 1) * 128)
                        qps = qkvps.tile([128, Q], F32, tag="qps")
                        for c in range(KC // 2):
                            nc.tensor.matmul(
                                qps[:], wq_sb[:, 2 * c:2 * c + 2, osl],
                                xn_sb[:, 2 * c:2 * c + 2, :Q],
                                start=(c == 0), stop=False,
                                perf_mode=DRM,
                            )
                        nc.tensor.matmul(
                            qps[:], augq_sb[:, :, osl], amr[:, :, :Q],
                            start=False, stop=True, perf_mode=DRM,
                        )
                        nc.scalar.activation(
                            q_sb[:, mc], qps[:], AF.Identity,
                            bias=bq8_sb[:, mc:mc + 1], scale=0.125 / 32.0,
                        )

                    for sc_ in range(KC):  # kv token chunks
                        ksl = slice(sc_ * 128, (sc_ + 1) * 128)
                        vps = qkvps.tile([128, R * HD], F32, tag="vps")
                        for c in range(KC // 2):
                            nc.tensor.matmul(
                                vps[:],
                                xn_sb[:, 2 * c:2 * c + 2, ksl],
                                wv_sb[:, 2 * c:2 * c + 2],
                                start=(c == 0), stop=False,
                                perf_mode=DRM,
                            )
                        nc.tensor.matmul(
                            vps[:], amr[:, :, ksl], augv_sb[:],
                            start=False, stop=True, perf_mode=DRM,
                        )
                        nc.vector.scalar_tensor_tensor(
                            v_sb[:, sc_], vps[:], 1.0 / 32.0, bv_sb[:],
                            OP.mult, OP.add,
                        )
                        ups = upsp.tile([128, 9], F32, tag="ups")
                        for c in range(KC // 2):
                            nc.tensor.matmul(
                                ups[:],
                                xn_sb[:, 2 * c:2 * c + 2, ksl],
                                wu8_sb[:, 2 * c:2 * c + 2],
                                start=(c == 0), stop=False,
                                perf_mode=DRM,
                            )
                        nc.tensor.matmul(
                            ups[:], amr[:, :, ksl], augu_sb[:],
                            start=False, stop=True, perf_mode=DRM,
                        )
                        nc.vector.scalar_tensor_tensor(
                            u8_sb[:, sc_], ups[:], 1.0 / 32.0, bu_sb[:],
                            OP.mult, OP.add,
                        )

            # ---------------- attention, per head ----------------
            # Rule-mix epilogue: GPSIMD (Pool) multiplies the attended
            # values (PSUM) by the per-query unnormalized rule weights
            # while copying to SBUF; DVE runs the bf16 add tree and a final
            # normalization scale (softmax over rules is scale-invariant,
            # so esum/einv stay off the dps->mix critical chain).
            attn_fmb = pp.tile([128, KC, Q], BF16)
            attn_fm = pp.tile([128, KC, Q], mybir.dt.float8e4)
            # FFN weights land during the attention phase (DMA is idle
            # there); the pool opens only after the LN1/QKV scratch frees
            wt_ctx = tc.tile_pool(name="wt", bufs=1)
            wt = wt_ctx.__enter__()
            wr1_sb = wt.tile([128, KC, 2 * D], mybir.dt.float8e4)
            wr2_sb = wt.tile([128, 2 * KC, D], mybir.dt.float8e4)
            for qtr in range(4):
                qs = slice(qtr * 512, (qtr + 1) * 512)
                nc.sync.dma_start(wr1_sb[:, :, qs], wr1_p[:, :, qs])
            for half in range(2):
                hs = slice(half * 512, (half + 1) * 512)
                nc.sync.dma_start(wr2_sb[:, :, hs], wr2_p[:, :, hs])
            with (
                tc.tile_pool(name="att", bufs=2) as att,
                tc.tile_pool(name="attw", bufs=3) as attw,
                tc.tile_pool(name="mwp", bufs=4) as mwp,
                tc.tile_pool(name="sps", bufs=2, space="PSUM") as spsp,
                tc.tile_pool(name="avps", bufs=2, space="PSUM") as avpsp,
                tc.tile_pool(name="dps", bufs=2, space="PSUM") as dpsp,
            ):
                DRM = mybir.MatmulPerfMode.DoubleRow
                for h in range(H):
                    kp, off = h // 2, 64 * (h % 2)
                    e_sb = att.tile([128, KC, Q], mybir.dt.float8e4, tag="e_sb")
                    for sc2 in range(KC // 2):
                        sps = spsp.tile([128, 2, Q], F32, tag="sps")
                        for j in range(2):
                            nc.tensor.matmul(
                                sps[:, j],
                                k_sb[off:off + 64, kp,
                                     (2 * sc2 + j) * 128:(2 * sc2 + j + 1) * 128],
                                q_sb[off:off + 64, kp, :],
                                start=True, stop=True,
                            )
                        nc.scalar.activation(e_sb[:, 2 * sc2:2 * sc2 + 2], sps[:], AF.Exp)

                    for pair in range(QC // 2):
                        av_pair = []
                        rinv = attw.tile([128, 2], F32, tag="rinv")
                        lsc = attw.tile([128, 2, R], BF16, tag="lsc")
                        for jq in range(2):
                            qc = 2 * pair + jq
                            qsl = slice(qc * 128, (qc + 1) * 128)
                            avps = avpsp.tile([128, R * HD], F32, tag="avps")
                            av_pair.append(avps)
                            for c in range(KC // 2):
                                nc.tensor.matmul(
                                    avps[:], e_sb[:, 2 * c:2 * c + 2, qsl],
                                    v_sb[:, 2 * c:2 * c + 2],
                                    start=(c == 0), stop=(c == KC // 2 - 1),
                                    perf_mode=DRM,
                                )
                            # rule logits (cols 0..7) + denominator (col 8)
                            # via the u-projection on the tensor engine
                            dps = dpsp.tile([128, 9], F32, tag="dps")
                            for c in range(KC // 2):
                                nc.tensor.matmul(
                                    dps[:], e_sb[:, 2 * c:2 * c + 2, qsl],
                                    u8_sb[:, 2 * c:2 * c + 2],
                                    start=(c == 0), stop=(c == KC // 2 - 1),
                                    perf_mode=DRM,
                                )
                            nc.vector.reciprocal(rinv[:, jq:jq + 1],
                                                 dps[:, 8:9])
                            nc.vector.tensor_scalar_mul(
                                lsc[:, jq], dps[:, 0:8], rinv[:, jq:jq + 1])
                        # one merged exp per qc pair halves the tiny-exp
                        # instruction overhead on the bottleneck ACT engine
                        elog = attw.tile([128, 2, R], BF16, tag="elog")
                        nc.scalar.activation(elog[:], lsc[:], AF.Exp)
                        esum = attw.tile([128, 2], F32, tag="esum")
                        nc.vector.reduce_sum(esum[:], elog[:],
                                             axis=mybir.AxisListType.X)
                        einv = attw.tile([128, 2], F32, tag="einv")
                        nc.vector.reciprocal(einv[:], esum[:])
                        s_q = attw.tile([128, 2], F32, tag="s_q")
                        nc.vector.tensor_mul(s_q[:], einv[:], rinv[:])
                        for jq in range(2):
                            qc = 2 * pair + jq
                            # GPSIMD cannot touch PSUM: DVE multiplies the
                            # raw attended values by the unnormalized rule
                            # weights straight from PSUM, then the add tree
                            mw = mwp.tile([128, R, HD], BF16, tag="mw")
                            nc.vector.tensor_tensor(
                                mw[:],
                                av_pair[jq][:].rearrange("p (r d) -> p r d",
                                                         d=HD),
                                elog[:, jq, :, None].to_broadcast(
                                    [128, R, HD]),
                                OP.mult,
                            )
                            t4 = attw.tile([128, 4, HD], BF16, tag="t4")
                            nc.vector.tensor_add(t4[:], mw[:, 0:4],
                                                 mw[:, 4:8])
                            t2 = attw.tile([128, 2, HD], BF16, tag="t2")
                            nc.gpsimd.tensor_add(t2[:], t4[:, 0:2],
                                                 t4[:, 2:4])
                            t1 = attw.tile([128, HD], BF16, tag="t1")
                            nc.vector.tensor_add(t1[:], t2[:, 0], t2[:, 1])
                            nc.vector.tensor_scalar_mul(
                                attn_sb[:, qc, h * HD:(h + 1) * HD], t1[:],
                                s_q[:, jq:jq + 1])
                    if h % 2 == 1:
                        # heads 2dc,2dc+1 fill feature chunk dc of attn_sb:
                        # transpose to feature-major on the (idle) DMA xbar
                        # while later heads run, then one fp8 downcast per
                        # chunk row for the DoubleRow Wf matmuls
                        dc = h // 2
                        for qc in range(QC):
                            nc.sync.dma_start_transpose(
                                attn_fmb[:, dc, qc * 128:(qc + 1) * 128],
                                attn_sb[:, qc, dc * 128:(dc + 1) * 128],
                            )
                        nc.gpsimd.tensor_copy(attn_fm[:, dc], attn_fmb[:, dc])

                # ---------- Wf + residual (inside the attention pools:
                # fps reuses the sps PSUM slots, avoiding the pool-close
                # barrier between attention and the tail) ----------
                for mc in range(KC):
                    osl = slice(mc * 128, (mc + 1) * 128)
                    if mc % 2 == 0:
                        fpt2 = spsp.tile([128, 2, Q], F32, tag="sps",
                                         name=f"fpt2_{mc}")
                        fpt = fpt2[:, 0]
                    else:
                        fpt = avpsp.tile([128, R * HD], F32, tag="avps",
                                         name=f"fpt_{mc}")
                    for c in range(KC // 2):
                        nc.tensor.matmul(
                            fpt[:], wf_sb[:, 2 * c:2 * c + 2, osl],
                            attn_fm[:, 2 * c:2 * c + 2, :],
                            start=(c == 0), stop=(c == KC // 2 - 1),
                            perf_mode=DRM,
                        )
                    # y2 = wf_out/32 + (x_q + bf)
                    nc.vector.scalar_tensor_tensor(
                        y2_sb[:, mc], fpt[:], 1.0 / 32.0, xqb[:, mc],
                        OP.mult, OP.add,
                    )

            with (
                tc.tile_pool(name="tail", bufs=1) as tail,
            ):
                # ---------------- LN2 ----------------
                # on8 = y*rstd2 in fp8; -mean2*rstd2 folded into FFN1 via
                # the amr2 row (g2/b2 folded host-side into Wr1/br1)
                FP8 = mybir.dt.float8e4
                on8_sb = tail.tile([128, KC, Q], FP8)
                y_bf = tail.tile([128, KC, Q], BF16)
                amr2 = tail.tile([1, 2, Q], FP8)
                nc.vector.memset(amr2[:, 1], 0.0)
                with (
                    tc.tile_pool(name="ln2", bufs=1) as ln2,
                    tc.tile_pool(name="ln2sq", bufs=2) as ln2sq,
                    tc.tile_pool(name="ln2ps", bufs=1, space="PSUM") as ln2ps,
                ):
                    s2_ps = ln2ps.tile([128, Q], F32)
                    q2_ps = ln2ps.tile([128, Q], F32)
                    for kc in range(KC):
                        nc.gpsimd.tensor_copy(y_bf[:, kc], y2_sb[:, kc])
                        sq_c = ln2sq.tile([128, Q], BF16, tag="sq2_c")
                        nc.vector.tensor_mul(sq_c[:], y_bf[:, kc], y_bf[:, kc])
                        nc.tensor.matmul(
                            s2_ps[:], ones_bf[:], y_bf[:, kc],
                            start=(kc == 0), stop=(kc == KC - 1),
                        )
                        nc.tensor.matmul(
                            q2_ps[:], ones_bf[:], sq_c[:],
                            start=(kc == 0), stop=(kc == KC - 1),
                        )
                    var2 = ln2.tile([128, Q], F32)
                    rstd2 = ln2.tile([128, Q], BF16)
                    m22 = ln2.tile([128, Q], F32)
                    nc.scalar.activation(m22[:], s2_ps[:], AF.Square,
                                         scale=D ** -0.5)
                    nc.vector.scalar_tensor_tensor(
                        var2[:], m22[:], -1.0, q2_ps[:], OP.mult, OP.add,
                    )
                    nc.scalar.activation(var2[:], var2[:], AF.Ln, bias=EPS,
                                         scale=1.0 / D)
                    nc.scalar.activation(rstd2[:], var2[:], AF.Exp, scale=-0.5)
                    nc.vector.scalar_tensor_tensor(
                        amr2[:, 0], s2_ps[0:1, :], -32.0 / D, rstd2[0:1, :],
                        OP.mult, OP.mult,
                    )
                    for kc in range(KC):
                        eng = nc.gpsimd if kc % 2 else nc.vector
                        eng.tensor_mul(on8_sb[:, kc], y_bf[:, kc],
                                       rstd2[:])

                # ---------------- FFN ----------------
                tps2_ctx = tc.tile_pool(name="tps2", bufs=4, space="PSUM")
                tps = tps2_ctx.__enter__()
                DRM = mybir.MatmulPerfMode.DoubleRow
                h_sb = tail.tile([128, 2 * KC, Q], FP8)
                for mc in range(2 * KC):
                    osl = slice(mc * 128, (mc + 1) * 128)
                    hps = tps.tile([128, Q], F32, tag="hps")
                    for c in range(KC // 2):
                        nc.tensor.matmul(
                            hps[:], wr1_sb[:, 2 * c:2 * c + 2, osl],
                            on8_sb[:, 2 * c:2 * c + 2, :],
                            start=(c == 0), stop=False,
                            perf_mode=DRM,
                        )
                    nc.tensor.matmul(
                        hps[:], augr1_sb[:, :, osl], amr2[:],
                        start=False, stop=True, perf_mode=DRM,
                    )
                    nc.scalar.activation(
                        h_sb[:, mc], hps[:], AF.Relu,
                        bias=br1_sb[:, mc:mc + 1], scale=1.0 / 32.0,
                    )

                out_sb = tail.tile([128, KC, Q], F32)
                for mc in range(KC):
                    osl = slice(mc * 128, (mc + 1) * 128)
                    ops = tps.tile([128, Q], F32, tag="ops")
                    for c in range(KC):
                        nc.tensor.matmul(
                            ops[:], wr2_sb[:, 2 * c:2 * c + 2, osl],
                            h_sb[:, 2 * c:2 * c + 2, :],
                            start=(c == 0), stop=(c == KC - 1),
                            perf_mode=DRM,
                        )
                    # out = (ffn2/32 + br2) + y2
                    nc.vector.tensor_scalar(
                        out_sb[:, mc], ops[:], 1.0 / 32.0,
                        br2_sb[:, mc:mc + 1], OP.mult, OP.add,
                    )
                    eng = nc.gpsimd if mc % 2 else nc.vector
                    eng.tensor_tensor(
                        out_sb[:, mc], out_sb[:, mc], y2_sb[:, mc], OP.add,
                    )
                    nc.sync.dma_start(
                        out_d[:, :].rearrange("(c p) t -> p c t", p=128)[:, mc],
                        out_sb[:, mc],
                    )
                tps2_ctx.__exit__(None, None, None)
            wt_ctx.__exit__(None, None, None)
            akv_ctx.__exit__(None, None, None)

    _split_multi_waits(nc)
    return nc


_NC_CACHE = None


def _get_nc():
    global _NC_CACHE
    if _NC_CACHE is None:
        _NC_CACHE = _build_nc()
    return _NC_CACHE


DIM_ = 1024


def kernel(x, Wq, bq, Wk, bk, Wv, bv, Wqv, bqv, Ws, bs, Wf, bf, Wr1, br1,
           Wr2, br2, g1, b1, g2, b2):
    x = np.asarray(x, dtype=np.float32)
    f32c = lambda a: np.ascontiguousarray(np.asarray(a), dtype=np.float32)
    import ml_dtypes
    bf16c = lambda a: np.ascontiguousarray(
        np.asarray(a, dtype=np.float32).astype(ml_dtypes.bfloat16))

    # fold the LN affine params into the consuming linears:
    #   LN1 g1/b1 -> Wq/Wk/Wv (and the derived u-projection) + biases
    #   LN2 g2/b2 -> Wr1/br1
    g1f = np.asarray(g1, np.float32)
    b1f = np.asarray(b1, np.float32)
    g2f = np.asarray(g2, np.float32)
    b2f = np.asarray(b2, np.float32)
    Wq_f = np.asarray(Wq, np.float32) * g1f[None, :]
    bq_f = np.asarray(bq, np.float32) + np.asarray(Wq, np.float32) @ b1f
    Wk_f = np.asarray(Wk, np.float32) * g1f[None, :]
    bk_f = np.asarray(bk, np.float32) + np.asarray(Wk, np.float32) @ b1f
    Wv_f = np.asarray(Wv, np.float32) * g1f[None, :]
    bv_f = np.asarray(bv, np.float32) + np.asarray(Wv, np.float32) @ b1f
    Wr1_f = np.asarray(Wr1, np.float32) * g2f[None, :]
    br1_f = np.asarray(br1, np.float32) + np.asarray(Wr1, np.float32) @ b2f

    ws_vec = np.asarray(Ws, np.float32)[0, 32:32 + HD]
    Wu = np.einsum("d,rdf->fr", ws_vec, Wv_f.reshape(R, HD, DIM_))
    bu = np.einsum("d,rd->r", ws_vec, bv_f.reshape(R, HD))
    wu8 = np.zeros((DIM_, 9), np.float32)
    wu8[:, 0:8] = Wu * 32.0
    bu_bc = np.zeros((128, 9), np.float32)
    bu_bc[:, 0:8] = bu[None, :]
    bu_bc[:, 8] = 1.0

    # rank-1 mean-correction rows: the device computes xn' = x*rstd and the
    # matmuls subtract mean*rstd via one extra accumulation with these
    # column-sum vectors (row-sums of the folded weights).
    def _pad2(v):
        # [2, n] with a zero second row: lets the rank-1 mean-correction
        # run as a DoubleRow fp8 matmul (0.5 cyc/row)
        m = np.zeros((2, v.shape[0]), np.float32)
        m[0] = v
        return np.ascontiguousarray(m.astype(ml_dtypes.float8_e4m3))
    aug_k = _pad2(Wk_f.sum(axis=1))
    aug_q = _pad2(Wq_f.sum(axis=1))
    aug_v = _pad2(Wv_f.sum(axis=1))
    aug_u = _pad2(wu8.sum(axis=0) / 32.0)
    aug_r1 = _pad2(Wr1_f.sum(axis=1))

    shared = {
        "wu8": np.ascontiguousarray(wu8.astype(ml_dtypes.float8_e4m3)),
        "bu_bc": bu_bc,
        "aug_k": aug_k, "aug_q": aug_q, "aug_v": aug_v, "aug_u": aug_u,
        "aug_r1": aug_r1,
        "wqT": np.ascontiguousarray((Wq_f.T * 32.0
                                     ).astype(ml_dtypes.float8_e4m3)),
        "wkT": np.ascontiguousarray((Wk_f.T * 32.0
                                     ).astype(ml_dtypes.float8_e4m3)),
        "wvT": np.ascontiguousarray((Wv_f.T * 32.0
                                     ).astype(ml_dtypes.float8_e4m3)),
        "wfT": np.ascontiguousarray((np.asarray(Wf, np.float32).T * 32.0
                                     ).astype(ml_dtypes.float8_e4m3)),
        "wr1T": np.ascontiguousarray(
            (Wr1_f.T * 32.0).astype(ml_dtypes.float8_e4m3)),
        "wr2T": np.ascontiguousarray(
            (np.asarray(Wr2, np.float32).T * 32.0).astype(
                ml_dtypes.float8_e4m3)),
        "bk": f32c(bk_f),
        "bq8": f32c(bq_f / 8.0),
        "bf": f32c(bf),
        "br1": f32c(br1_f),
        "br2": f32c(br2),
        "bv_bc": f32c(np.tile(bv_f[None, :], (128, 1))),
        "ones_c": np.ones((128, 128), dtype=np.float32),
        "ident_c": np.eye(128, dtype=np.float32),
        "consts_c": np.tile(np.array([[0.0, EPS]], dtype=np.float32), (128, 1)),
    }

    in_maps = []
    for c in range(8):
        b, half = c // 2, c % 2
        xb = x[b]
        x_rot = np.concatenate(
            [xb[half * Q:(half + 1) * Q], xb[(1 - half) * Q:(2 - half) * Q]], axis=0
        )
        m = dict(shared)
        m["x_fm"] = f32c(x_rot.T)
        in_maps.append(m)

    res = run_bass_kernel_spmd(_get_nc(), in_maps, core_ids=list(range(8)))

    out = np.empty((4, S, D), dtype=np.float32)
    for c in range(8):
        b, half = c // 2, c % 2
        out[b, half * Q:(half + 1) * Q, :] = res.results[c]["out_fm"].T
    return out



# revision 81
# speedup vs baseline: 1.0545x; 1.0545x over previous
"""Compositional-attention transformer block on 8 Trainium2 NeuronCores.

Sharding: core c in 0..7 handles batch b = c//2 and query-token half
h = c%2 (512 of 1024 tokens). Keys/values span the whole batch, so each
core computes LN + k/v over all 1024 tokens of its batch (duplicated
between the core pair) and q/attention/FFN for its 512 query tokens.
No collectives are needed.

Device layout is feature-major ([D, tokens]) so every linear's output
feeds the next matmul without transposes; LN statistics over the
partition (feature) axis are computed with an all-ones matmul that
leaves the per-token sums broadcast across all 128 partitions.

Key optimizations over the straightforward version:
- All weights are prefetched to SBUF during the LN1/x-load window, so
  no phase waits on a weight DMA.
- Both LayerNorms are folded into the consuming matmuls: the device
  computes xn' = x*rstd only, and the -mean*rstd correction enters each
  projection as one extra rank-1 accumulation (aug_* host vectors x the
  runtime amr row). g/b LN affines are folded host-side.
- Attention epilogue: the rule-selection softmax is applied
  UNNORMALIZED (softmax over R is scale/shift-invariant): exp(dps*rinv)
  weights multiply the raw attended values straight out of PSUM on DVE,
  a bf16 add-tree (split DVE/GPSIMD) reduces over rules, and one final
  per-query scale applies einv*rinv off the critical chain.
- attn token->feature transposes run on the DMA xbar
  (dma_start_transpose) during attention; GPSIMD handles SBUF-only
  copies/adds (it cannot touch PSUM - hardware constraint).

Math note: in the reference, the qv/Wqv/Ws[:, :32]/bs terms are
constant along the rule axis R, so they cancel inside the rule softmax
(softmax over R is shift-invariant). That whole pathway is dead code
and is not computed here.
"""

import sys

if "/opt/trn_rl_repo" not in sys.path:
    sys.path.insert(0, "/opt/trn_rl_repo")

import numpy as np

import bass_rust
import concourse.bass as bass
import concourse.mybir as mybir
from concourse.tile import TileContext
from concourse.bass_utils import run_bass_kernel_spmd

F32 = mybir.dt.float32
BF16 = mybir.dt.bfloat16

D = 1024      # model dim
S = 1024      # kv tokens per batch
Q = 512       # query tokens per core
H = 16        # heads
HD = 64       # head dim
R = 8         # rules
KC = D // 128   # 8 k-chunks of 128
QC = Q // 128   # 4 query chunks of 128
EPS = 1e-5


class _TC(TileContext):
    """TileContext whose tail drain splits its sem waits across NOPs.

    The walrus build in this environment accepts at most one sync-wait
    per CTRL instruction, so the stock single multi-wait drain fails
    codegen ("Too many sync wait commands")."""

    def _drain_and_barrier(self, tick_clock, wait_clock):
        gc = tick_clock.global_clock
        for p, t in enumerate(list(gc)):
            if t <= 0:
                continue
            sc = bass_rust.ScopedClock()
            sc.require_at_least(None, p, t)
            nop = self.nc.sync.nop(nofuse=True, hint="tail_wait")
            wait_clock.add_sem_waits(nop.ins, sc)
        self.nc.sync.drain()
        self.nc.all_engine_barrier()
        assert self.sems is not None
        popped = self.nc._tile_sem_poison_stack.pop()
        assert popped is self._sem_poison
        self.nc.clear_and_free_semaphores(list(self.sems.allocated().values()))
        self.nc.all_engine_barrier()


def _split_multi_waits(nc):
    """walrus here caps sync waits at 1 per instruction: hoist extras onto
    single-wait EventSemaphore instructions inserted just before, on the
    same engine (engine program order preserves the wait semantics)."""
    n = 0
    for fn in nc.m.functions:
        for bb in fn.blocks:
            out = []
            changed = False
            for inst in bb.instructions:
                si = inst.sync_info
                if si is not None and len(si.on_wait) > 1:
                    waits = list(si.on_wait)
                    for w in waits[:-1]:
                        ev = mybir.InstEventSemaphore(
                            name=f"I-wsp{nc.next_id()}", ins=[], outs=[],
                            sync_info=mybir.SyncInfo(on_wait=[w], on_update=[]),
                        )
                        ev.engine = inst.engine
                        out.append(ev)
                        n += 1
                    inst.sync_info = mybir.SyncInfo(
                        on_wait=[waits[-1]], on_update=list(si.on_update)
                    )
                    changed = True
                out.append(inst)
            if changed:
                bb.instructions = out
    return n


def _build_nc():
    nc = bass.Bass(target_bir_lowering=False, trn_type="TRN2")
    AF = mybir.ActivationFunctionType
    OP = mybir.AluOpType

    x_d = nc.dram_tensor("x_fm", [D, S], F32, kind="ExternalInput")
    wq_d = nc.dram_tensor("wqT", [D, D], mybir.dt.float8e4, kind="ExternalInput")
    wk_d = nc.dram_tensor("wkT", [D, D], mybir.dt.float8e4, kind="ExternalInput")
    wv_d = nc.dram_tensor("wvT", [D, R * HD], mybir.dt.float8e4, kind="ExternalInput")
    wf_d = nc.dram_tensor("wfT", [D, D], mybir.dt.float8e4, kind="ExternalInput")
    wr1_d = nc.dram_tensor("wr1T", [D, 2 * D], mybir.dt.float8e4, kind="ExternalInput")
    wr2_d = nc.dram_tensor("wr2T", [2 * D, D], mybir.dt.float8e4, kind="ExternalInput")
    bk_d = nc.dram_tensor("bk", [D], F32, kind="ExternalInput")
    bq8_d = nc.dram_tensor("bq8", [D], F32, kind="ExternalInput")
    bf_d = nc.dram_tensor("bf", [D], F32, kind="ExternalInput")
    br1_d = nc.dram_tensor("br1", [2 * D], F32, kind="ExternalInput")
    br2_d = nc.dram_tensor("br2", [D], F32, kind="ExternalInput")
    bv_d = nc.dram_tensor("bv_bc", [128, R * HD], F32, kind="ExternalInput")
    wu8_d = nc.dram_tensor("wu8", [D, 9], mybir.dt.float8e4, kind="ExternalInput")
    bu_d = nc.dram_tensor("bu_bc", [128, 9], F32, kind="ExternalInput")
    FP8 = mybir.dt.float8e4
    augk_d = nc.dram_tensor("aug_k", [2, D], FP8, kind="ExternalInput")
    augq_d = nc.dram_tensor("aug_q", [2, D], FP8, kind="ExternalInput")
    augv_d = nc.dram_tensor("aug_v", [2, R * HD], FP8, kind="ExternalInput")
    augu_d = nc.dram_tensor("aug_u", [2, 9], FP8, kind="ExternalInput")
    augr1_d = nc.dram_tensor("aug_r1", [2, 2 * D], FP8, kind="ExternalInput")
    ones_d = nc.dram_tensor("ones_c", [128, 128], F32, kind="ExternalInput")
    ident_d = nc.dram_tensor("ident_c", [128, 128], F32, kind="ExternalInput")
    consts_d = nc.dram_tensor("consts_c", [128, 2], F32, kind="ExternalInput")
    out_d = nc.dram_tensor("out_fm", [D, Q], F32, kind="ExternalOutput")

    x_pct = x_d[:, :].rearrange("(c p) t -> p c t", p=128)

    with _TC(nc) as tc:
        with tc.tile_pool(name="persist", bufs=1) as pp:
            xqb = pp.tile([128, KC, Q], F32)
            ones_sb = pp.tile([128, 128], F32)
            consts_sb = pp.tile([128, 2], F32)
            # register constants used as implicit activation biases
            nc.const_aps.aps[(F32, 0.0)] = consts_sb[:, 0:1]
            nc.const_aps.aps[(F32, EPS)] = consts_sb[:, 1:2]
            bk_sb = pp.tile([128, KC], F32)
            bq8_sb = pp.tile([128, KC], F32)
            bf_sb = pp.tile([128, KC], F32)
            br1_sb = pp.tile([128, 2 * KC], F32)
            br2_sb = pp.tile([128, KC], F32)
            bv_sb = pp.tile([128, R * HD], F32)
            bu_sb = pp.tile([128, 9], F32)

            wq_p = wq_d[:, :].rearrange("(c p) o -> p c o", p=128)
            wk_p = wk_d[:, :].rearrange("(c p) o -> p c o", p=128)
            wv_p = wv_d[:, :].rearrange("(c p) o -> p c o", p=128)
            wf_p = wf_d[:, :].rearrange("(c p) o -> p c o", p=128)
            wr1_p = wr1_d[:, :].rearrange("(c p) o -> p c o", p=128)
            wr2_p = wr2_d[:, :].rearrange("(c p) o -> p c o", p=128)
            wk_sb = pp.tile([128, KC, D], mybir.dt.float8e4)
            wq_sb = pp.tile([128, KC, D], mybir.dt.float8e4)
            wv_sb = pp.tile([128, KC, R * HD], mybir.dt.float8e4)
            wu8_sb = pp.tile([128, KC, 9], mybir.dt.float8e4)
            wf_sb = pp.tile([128, KC, D], mybir.dt.float8e4)

            augk_sb = pp.tile([1, 2, D], mybir.dt.float8e4)
            augq_sb = pp.tile([1, 2, D], mybir.dt.float8e4)
            augv_sb = pp.tile([1, 2, R * HD], mybir.dt.float8e4)
            augu_sb = pp.tile([1, 2, 9], mybir.dt.float8e4)
            augr1_sb = pp.tile([1, 2, 2 * D], mybir.dt.float8e4)
            ones_bf = pp.tile([128, 128], BF16)

            attn_sb = pp.tile([128, QC, D], BF16)   # token-major attn output
            y2_sb = pp.tile([128, KC, Q], F32)     # post-Wf residual stream

            # k/q/v live from QKV until end of attention
            akv_ctx = tc.tile_pool(name="akv", bufs=1)
            akv = akv_ctx.__enter__()
            k_sb = akv.tile([128, KC, S], BF16)
            q_sb = akv.tile([128, KC, Q], BF16)
            v_sb = akv.tile([128, KC, R * HD], mybir.dt.float8e4)
            u8_sb = akv.tile([128, KC, 9], mybir.dt.float8e4)

            with tc.tile_pool(name="mid", bufs=1) as mid:
                # tiny consts first (the stats matmuls need ones_bf), then
                # x — every compute chain waits on it — then weights
                nc.sync.dma_start(ones_sb[:], ones_d[:, :])
                nc.sync.dma_start(consts_sb[:], consts_d[:, :])
                nc.vector.tensor_copy(ones_bf[:], ones_sb[:])
                xfm = mid.tile([128, KC, S], F32)
                for kc in range(KC):
                    nc.sync.dma_start(xfm[:, kc], x_pct[:, kc])
                for half in range(2):
                    hs = slice(half * 512, (half + 1) * 512)
                    nc.sync.dma_start(wk_sb[:, :, hs], wk_p[:, :, hs])
                    nc.sync.dma_start(wq_sb[:, :, hs], wq_p[:, :, hs])
                nc.sync.dma_start(wv_sb[:], wv_p)
                nc.sync.dma_start(
                    wu8_sb[:], wu8_d[:, :].rearrange("(c p) o -> p c o", p=128))
                nc.sync.dma_start(augk_sb[:], augk_d[:, :].rearrange("(a r) o -> a r o", a=1))
                nc.sync.dma_start(augq_sb[:], augq_d[:, :].rearrange("(a r) o -> a r o", a=1))
                nc.sync.dma_start(augv_sb[:], augv_d[:, :].rearrange("(a r) o -> a r o", a=1))
                nc.sync.dma_start(augu_sb[:], augu_d[:, :].rearrange("(a r) o -> a r o", a=1))
                nc.sync.dma_start(bk_sb[:], bk_d[:].rearrange("(c p) -> p c", p=128))
                nc.sync.dma_start(bq8_sb[:], bq8_d[:].rearrange("(c p) -> p c", p=128))
                nc.sync.dma_start(bf_sb[:], bf_d[:].rearrange("(c p) -> p c", p=128))
                nc.sync.dma_start(bv_sb[:], bv_d[:, :])
                nc.sync.dma_start(bu_sb[:], bu_d[:, :])
                nc.sync.dma_start(br1_sb[:], br1_d[:].rearrange("(c p) -> p c", p=128))
                nc.sync.dma_start(br2_sb[:], br2_d[:].rearrange("(c p) -> p c", p=128))
                nc.sync.dma_start(augr1_sb[:], augr1_d[:, :].rearrange("(a r) o -> a r o", a=1))
                for half in range(2):
                    hs = slice(half * 512, (half + 1) * 512)
                    nc.sync.dma_start(wf_sb[:, :, hs], wf_p[:, :, hs])

                x_bf = mid.tile([128, KC, S], BF16)
                amr = mid.tile([1, 2, S], mybir.dt.float8e4)
                nc.vector.memset(amr[:, 1], 0.0)
                for mc in range(KC):
                    nc.gpsimd.tensor_scalar_add(
                        xqb[:, mc], xfm[:, mc, :Q], bf_sb[:, mc:mc + 1])
                xn_sb = mid.tile([128, KC, S], mybir.dt.float8e4)

                # ---------------- LN1 (feature-major) ----------------
                # xn' = x*rstd only; the -mean*rstd correction is folded
                # into every projection as one extra rank-1 matmul with the
                # amr row (see aug_* host vectors).
                with (
                    tc.tile_pool(name="ln1", bufs=1) as ln1,
                    tc.tile_pool(name="ln1sq", bufs=2) as ln1sq,
                    tc.tile_pool(name="ln1ps", bufs=1, space="PSUM") as ln1ps,
                ):
                    s_ps = [ln1ps.tile([128, 512], F32, name=f"s_ps{i}") for i in range(2)]
                    q_ps = [ln1ps.tile([128, 512], F32, name=f"q_ps{i}") for i in range(2)]
                    for kc in range(KC):
                        nc.vector.tensor_copy(x_bf[:, kc], xfm[:, kc])
                        sq_c = ln1sq.tile([128, S], BF16, tag="sq_c")
                        nc.vector.tensor_mul(sq_c[:], x_bf[:, kc], x_bf[:, kc])
                        for nh in range(2):
                            nc.tensor.matmul(
                                s_ps[nh][:], ones_bf[:],
                                x_bf[:, kc, nh * 512:(nh + 1) * 512],
                                start=(kc == 0), stop=(kc == KC - 1),
                            )
                            nc.tensor.matmul(
                                q_ps[nh][:], ones_bf[:],
                                sq_c[:, nh * 512:(nh + 1) * 512],
                                start=(kc == 0), stop=(kc == KC - 1),
                            )

                    # process per token-half so xn(half0) lands while half1's
                    # var chain still runs; D*var = q - s^2/D, with the /D
                    # folded into the Ln scale
                    rstd_sb = ln1.tile([128, S], BF16)
                    var_sb = ln1.tile([128, S], F32)
                    m2_sb = ln1.tile([128, S], F32)
                    for nh in range(2):
                        sl = slice(nh * 512, (nh + 1) * 512)
                        # m2 = s^2/D on ACT (DVE cannot read 2 PSUM inputs)
                        nc.scalar.activation(m2_sb[:, sl], s_ps[nh][:],
                                             AF.Square, scale=D ** -0.5)
                        nc.vector.scalar_tensor_tensor(
                            var_sb[:, sl], m2_sb[:, sl], -1.0,
                            q_ps[nh][:], OP.mult, OP.add,
                        )
                        # rstd = exp(-0.5*ln(var/D+eps))
                        nc.scalar.activation(var_sb[:, sl], var_sb[:, sl],
                                             AF.Ln, bias=EPS, scale=1.0 / D)
                        nc.scalar.activation(rstd_sb[:, sl], var_sb[:, sl],
                                             AF.Exp, scale=-0.5)
                        # amr = -32*mean*rstd = -(32/D)*s*rstd (fp8 row)
                        nc.vector.scalar_tensor_tensor(
                            amr[:, 0, sl], s_ps[nh][0:1, :], -32.0 / D,
                            rstd_sb[0:1, sl], OP.mult, OP.mult,
                        )
                        for kc in range(KC):
                            nc.vector.tensor_mul(xn_sb[:, kc, sl],
                                                 x_bf[:, kc, sl],
                                                 rstd_sb[:, sl])

                # ---------------- q, k, v linears ----------------
                with (
                    tc.tile_pool(name="qkvps", bufs=2, space="PSUM") as qkvps,
                    tc.tile_pool(name="upsp", bufs=2, space="PSUM") as upsp,
                ):
                    DRM = mybir.MatmulPerfMode.DoubleRow
                    for mc in range(KC):
                        osl = slice(mc * 128, (mc + 1) * 128)
                        for nh in range(2):
                            hsl = slice(nh * 512, (nh + 1) * 512)
                            kps = qkvps.tile([128, 512], F32, tag="kps")
                            for c in range(KC // 2):
                                nc.tensor.matmul(
                                    kps[:], wk_sb[:, 2 * c:2 * c + 2, osl],
                                    xn_sb[:, 2 * c:2 * c + 2, hsl],
                                    start=(c == 0), stop=False,
                                    perf_mode=DRM,
                                )
                            nc.tensor.matmul(
                                kps[:], augk_sb[:, :, osl], amr[:, :, hsl],
                                start=False, stop=True, perf_mode=DRM,
                            )
                            nc.scalar.activation(
                                k_sb[:, mc, hsl], kps[:],
                                AF.Identity, bias=bk_sb[:, mc:mc + 1],
                                scale=1.0 / 32.0,
                            )

                    for mc in range(KC):
                        osl = slice(mc * 128, (mc + 1) * 128)
                        qps = qkvps.tile([128, Q], F32, tag="qps")
                        for c in range(KC // 2):
                            nc.tensor.matmul(
                                qps[:], wq_sb[:, 2 * c:2 * c + 2, osl],
                                xn_sb[:, 2 * c:2 * c + 2, :Q],
                                start=(c == 0), stop=False,
                                perf_mode=DRM,
                            )
                        nc.tensor.matmul(
                            qps[:], augq_sb[:, :, osl], amr[:, :, :Q],
                            start=False, stop=True, perf_mode=DRM,
                        )
                        nc.scalar.activation(
                            q_sb[:, mc], qps[:], AF.Identity,
                            bias=bq8_sb[:, mc:mc + 1], scale=0.125 / 32.0,
                        )

                    for sc_ in range(KC):  # kv token chunks
                        ksl = slice(sc_ * 128, (sc_ + 1) * 128)
                        vps = qkvps.tile([128, R * HD], F32, tag="vps")
                        for c in range(KC // 2):
                            nc.tensor.matmul(
                                vps[:],
                                xn_sb[:, 2 * c:2 * c + 2, ksl],
                                wv_sb[:, 2 * c:2 * c + 2],
                                start=(c == 0), stop=False,
                                perf_mode=DRM,
                            )
                        nc.tensor.matmul(
                            vps[:], amr[:, :, ksl], augv_sb[:],
                            start=False, stop=True, perf_mode=DRM,
                        )
                        nc.vector.scalar_tensor_tensor(
                            v_sb[:, sc_], vps[:], 1.0 / 32.0, bv_sb[:],
                            OP.mult, OP.add,
                        )
                        ups = upsp.tile([128, 9], F32, tag="ups")
                        for c in range(KC // 2):
                            nc.tensor.matmul(
                                ups[:],
                                xn_sb[:, 2 * c:2 * c + 2, ksl],
                                wu8_sb[:, 2 * c:2 * c + 2],
                                start=(c == 0), stop=False,
                                perf_mode=DRM,
                            )
                        nc.tensor.matmul(
                            ups[:], amr[:, :, ksl], augu_sb[:],
                            start=False, stop=True, perf_mode=DRM,
                        )
                        nc.vector.scalar_tensor_tensor(
                            u8_sb[:, sc_], ups[:], 1.0 / 32.0, bu_sb[:],
                            OP.mult, OP.add,
                        )

            # ---------------- attention, per head ----------------
            # Rule-mix epilogue: GPSIMD (Pool) multiplies the attended
            # values (PSUM) by the per-query unnormalized rule weights
            # while copying to SBUF; DVE runs the bf16 add tree and a final
            # normalization scale (softmax over rules is scale-invariant,
            # so esum/einv stay off the dps->mix critical chain).
            attn_fmb = pp.tile([128, KC, Q], BF16)
            attn_fm = pp.tile([128, KC, Q], mybir.dt.float8e4)
            # FFN weights land during the attention phase (DMA is idle
            # there); the pool opens only after the LN1/QKV scratch frees
            wt_ctx = tc.tile_pool(name="wt", bufs=1)
            wt = wt_ctx.__enter__()
            wr1_sb = wt.tile([128, KC, 2 * D], mybir.dt.float8e4)
            wr2_sb = wt.tile([128, 2 * KC, D], mybir.dt.float8e4)
            for qtr in range(4):
                qs = slice(qtr * 512, (qtr + 1) * 512)
                nc.sync.dma_start(wr1_sb[:, :, qs], wr1_p[:, :, qs])
            for half in range(2):
                hs = slice(half * 512, (half + 1) * 512)
                nc.sync.dma_start(wr2_sb[:, :, hs], wr2_p[:, :, hs])
            with (
                tc.tile_pool(name="att", bufs=2) as att,
                tc.tile_pool(name="attw", bufs=3) as attw,
                tc.tile_pool(name="mwp", bufs=4) as mwp,
                tc.tile_pool(name="sps", bufs=2, space="PSUM") as spsp,
                tc.tile_pool(name="avps", bufs=2, space="PSUM") as avpsp,
                tc.tile_pool(name="dps", bufs=2, space="PSUM") as dpsp,
            ):
                DRM = mybir.MatmulPerfMode.DoubleRow
                for h in range(H):
                    kp, off = h // 2, 64 * (h % 2)
                    e_sb = att.tile([128, KC, Q], mybir.dt.float8e4, tag="e_sb")
                    for sc2 in range(KC // 2):
                        sps = spsp.tile([128, 2, Q], F32, tag="sps")
                        for j in range(2):
                            nc.tensor.matmul(
                                sps[:, j],
                                k_sb[off:off + 64, kp,
                                     (2 * sc2 + j) * 128:(2 * sc2 + j + 1) * 128],
                                q_sb[off:off + 64, kp, :],
                                start=True, stop=True,
                            )
                        nc.scalar.activation(e_sb[:, 2 * sc2:2 * sc2 + 2], sps[:], AF.Exp)

                    for qc in range(QC):
                        qsl = slice(qc * 128, (qc + 1) * 128)
                        avps = avpsp.tile([128, R * HD], F32, tag="avps")
                        for c in range(KC // 2):
                            nc.tensor.matmul(
                                avps[:], e_sb[:, 2 * c:2 * c + 2, qsl],
                                v_sb[:, 2 * c:2 * c + 2],
                                start=(c == 0), stop=(c == KC // 2 - 1),
                                perf_mode=DRM,
                            )
                        # rule logits (cols 0..7) + denominator (col 8)
                        # via the u-projection on the tensor engine
                        dps = dpsp.tile([128, 9], F32, tag="dps")
                        for c in range(KC // 2):
                            nc.tensor.matmul(
                                dps[:], e_sb[:, 2 * c:2 * c + 2, qsl],
                                u8_sb[:, 2 * c:2 * c + 2],
                                start=(c == 0), stop=(c == KC // 2 - 1),
                                perf_mode=DRM,
                            )
                        rinv = attw.tile([128, 1], F32, tag="rinv")
                        nc.vector.reciprocal(rinv[:], dps[:, 8:9])
                        # unnormalized rule weights straight from PSUM
                        elog = attw.tile([128, R], BF16, tag="elog")
                        nc.scalar.activation(elog[:], dps[:, 0:8], AF.Exp,
                                             scale=rinv[:])
                        esum = attw.tile([128, 1], F32, tag="esum")
                        nc.vector.reduce_sum(esum[:], elog[:],
                                             axis=mybir.AxisListType.X)
                        einv = attw.tile([128, 1], F32, tag="einv")
                        nc.vector.reciprocal(einv[:], esum[:])
                        s_q = attw.tile([128, 1], F32, tag="s_q")
                        nc.vector.tensor_mul(s_q[:], einv[:], rinv[:])
                        # GPSIMD cannot touch PSUM: DVE multiplies the raw
                        # attended values by the unnormalized rule weights
                        # straight from PSUM, then runs the bf16 add tree
                        mw = mwp.tile([128, R, HD], BF16, tag="mw")
                        nc.vector.tensor_tensor(
                            mw[:],
                            avps[:].rearrange("p (r d) -> p r d", d=HD),
                            elog[:, :, None].to_broadcast([128, R, HD]),
                            OP.mult,
                        )
                        t4 = attw.tile([128, 4, HD], BF16, tag="t4")
                        nc.vector.tensor_add(t4[:], mw[:, 0:4], mw[:, 4:8])
                        t2 = attw.tile([128, 2, HD], BF16, tag="t2")
                        nc.gpsimd.tensor_add(t2[:], t4[:, 0:2], t4[:, 2:4])
                        t1 = attw.tile([128, HD], BF16, tag="t1")
                        nc.vector.tensor_add(t1[:], t2[:, 0], t2[:, 1])
                        nc.vector.tensor_scalar_mul(
                            attn_sb[:, qc, h * HD:(h + 1) * HD], t1[:], s_q[:])
                    if h % 2 == 1:
                        # heads 2dc,2dc+1 fill feature chunk dc of attn_sb:
                        # transpose to feature-major on the (idle) DMA xbar
                        # while later heads run, then one fp8 downcast per
                        # chunk row for the DoubleRow Wf matmuls
                        dc = h // 2
                        for qc in range(QC):
                            nc.sync.dma_start_transpose(
                                attn_fmb[:, dc, qc * 128:(qc + 1) * 128],
                                attn_sb[:, qc, dc * 128:(dc + 1) * 128],
                            )
                        nc.gpsimd.tensor_copy(attn_fm[:, dc], attn_fmb[:, dc])

                # ---------- Wf + residual (inside the attention pools:
                # fps reuses the sps PSUM slots, avoiding the pool-close
                # barrier between attention and the tail) ----------
                for mc in range(KC):
                    osl = slice(mc * 128, (mc + 1) * 128)
                    if mc % 2 == 0:
                        fpt2 = spsp.tile([128, 2, Q], F32, tag="sps",
                                         name=f"fpt2_{mc}")
                        fpt = fpt2[:, 0]
                    else:
                        fpt = avpsp.tile([128, R * HD], F32, tag="avps",
                                         name=f"fpt_{mc}")
                    for c in range(KC // 2):
                        nc.tensor.matmul(
                            fpt[:], wf_sb[:, 2 * c:2 * c + 2, osl],
                            attn_fm[:, 2 * c:2 * c + 2, :],
                            start=(c == 0), stop=(c == KC // 2 - 1),
                            perf_mode=DRM,
                        )
                    # y2 = wf_out/32 + (x_q + bf)
                    nc.vector.scalar_tensor_tensor(
                        y2_sb[:, mc], fpt[:], 1.0 / 32.0, xqb[:, mc],
                        OP.mult, OP.add,
                    )

            with (
                tc.tile_pool(name="tail", bufs=1) as tail,
            ):
                # ---------------- LN2 ----------------
                # on8 = y*rstd2 in fp8; -mean2*rstd2 folded into FFN1 via
                # the amr2 row (g2/b2 folded host-side into Wr1/br1)
                FP8 = mybir.dt.float8e4
                on8_sb = tail.tile([128, KC, Q], FP8)
                y_bf = tail.tile([128, KC, Q], BF16)
                amr2 = tail.tile([1, 2, Q], FP8)
                nc.vector.memset(amr2[:, 1], 0.0)
                with (
                    tc.tile_pool(name="ln2", bufs=1) as ln2,
                    tc.tile_pool(name="ln2sq", bufs=2) as ln2sq,
                    tc.tile_pool(name="ln2ps", bufs=1, space="PSUM") as ln2ps,
                ):
                    s2_ps = ln2ps.tile([128, Q], F32)
                    q2_ps = ln2ps.tile([128, Q], F32)
                    for kc in range(KC):
                        nc.gpsimd.tensor_copy(y_bf[:, kc], y2_sb[:, kc])
                        sq_c = ln2sq.tile([128, Q], BF16, tag="sq2_c")
                        nc.vector.tensor_mul(sq_c[:], y_bf[:, kc], y_bf[:, kc])
                        nc.tensor.matmul(
                            s2_ps[:], ones_bf[:], y_bf[:, kc],
                            start=(kc == 0), stop=(kc == KC - 1),
                        )
                        nc.tensor.matmul(
                            q2_ps[:], ones_bf[:], sq_c[:],
                            start=(kc == 0), stop=(kc == KC - 1),
                        )
                    var2 = ln2.tile([128, Q], F32)
                    rstd2 = ln2.tile([128, Q], BF16)
                    m22 = ln2.tile([128, Q], F32)
                    nc.scalar.activation(m22[:], s2_ps[:], AF.Square,
                                         scale=D ** -0.5)
                    nc.vector.scalar_tensor_tensor(
                        var2[:], m22[:], -1.0, q2_ps[:], OP.mult, OP.add,
                    )
                    nc.scalar.activation(var2[:], var2[:], AF.Ln, bias=EPS,
                                         scale=1.0 / D)
                    nc.scalar.activation(rstd2[:], var2[:], AF.Exp, scale=-0.5)
                    nc.vector.scalar_tensor_tensor(
                        amr2[:, 0], s2_ps[0:1, :], -32.0 / D, rstd2[0:1, :],
                        OP.mult, OP.mult,
                    )
                    for kc in range(KC):
                        eng = nc.gpsimd if kc % 2 else nc.vector
                        eng.tensor_mul(on8_sb[:, kc], y_bf[:, kc],
                                       rstd2[:])

                # ---------------- FFN ----------------
                tps2_ctx = tc.tile_pool(name="tps2", bufs=4, space="PSUM")
                tps = tps2_ctx.__enter__()
                DRM = mybir.MatmulPerfMode.DoubleRow
                h_sb = tail.tile([128, 2 * KC, Q], FP8)
                for mc in range(2 * KC):
                    osl = slice(mc * 128, (mc + 1) * 128)
                    hps = tps.tile([128, Q], F32, tag="hps")
                    for c in range(KC // 2):
                        nc.tensor.matmul(
                            hps[:], wr1_sb[:, 2 * c:2 * c + 2, osl],
                            on8_sb[:, 2 * c:2 * c + 2, :],
                            start=(c == 0), stop=False,
                            perf_mode=DRM,
                        )
                    nc.tensor.matmul(
                        hps[:], augr1_sb[:, :, osl], amr2[:],
                        start=False, stop=True, perf_mode=DRM,
                    )
                    nc.scalar.activation(
                        h_sb[:, mc], hps[:], AF.Relu,
                        bias=br1_sb[:, mc:mc + 1], scale=1.0 / 32.0,
                    )

                out_sb = tail.tile([128, KC, Q], F32)
                for mc in range(KC):
                    osl = slice(mc * 128, (mc + 1) * 128)
                    ops = tps.tile([128, Q], F32, tag="ops")
                    for c in range(KC):
                        nc.tensor.matmul(
                            ops[:], wr2_sb[:, 2 * c:2 * c + 2, osl],
                            h_sb[:, 2 * c:2 * c + 2, :],
                            start=(c == 0), stop=(c == KC - 1),
                            perf_mode=DRM,
                        )
                    # out = (ffn2/32 + br2) + y2
                    nc.vector.tensor_scalar(
                        out_sb[:, mc], ops[:], 1.0 / 32.0,
                        br2_sb[:, mc:mc + 1], OP.mult, OP.add,
                    )
                    eng = nc.gpsimd if mc % 2 else nc.vector
                    eng.tensor_tensor(
                        out_sb[:, mc], out_sb[:, mc], y2_sb[:, mc], OP.add,
                    )
                    nc.sync.dma_start(
                        out_d[:, :].rearrange("(c p) t -> p c t", p=128)[:, mc],
                        out_sb[:, mc],
                    )
                tps2_ctx.__exit__(None, None, None)
            wt_ctx.__exit__(None, None, None)
            akv_ctx.__exit__(None, None, None)

    _split_multi_waits(nc)
    return nc


_NC_CACHE = None


def _get_nc():
    global _NC_CACHE
    if _NC_CACHE is None:
        _NC_CACHE = _build_nc()
    return _NC_CACHE


DIM_ = 1024


def kernel(x, Wq, bq, Wk, bk, Wv, bv, Wqv, bqv, Ws, bs, Wf, bf, Wr1, br1,
           Wr2, br2, g1, b1, g2, b2):
    x = np.asarray(x, dtype=np.float32)
    f32c = lambda a: np.ascontiguousarray(np.asarray(a), dtype=np.float32)
    import ml_dtypes
    bf16c = lambda a: np.ascontiguousarray(
        np.asarray(a, dtype=np.float32).astype(ml_dtypes.bfloat16))

    # fold the LN affine params into the consuming linears:
    #   LN1 g1/b1 -> Wq/Wk/Wv (and the derived u-projection) + biases
    #   LN2 g2/b2 -> Wr1/br1
    g1f = np.asarray(g1, np.float32)
    b1f = np.asarray(b1, np.float32)
    g2f = np.asarray(g2, np.float32)
    b2f = np.asarray(b2, np.float32)
    Wq_f = np.asarray(Wq, np.float32) * g1f[None, :]
    bq_f = np.asarray(bq, np.float32) + np.asarray(Wq, np.float32) @ b1f
    Wk_f = np.asarray(Wk, np.float32) * g1f[None, :]
    bk_f = np.asarray(bk, np.float32) + np.asarray(Wk, np.float32) @ b1f
    Wv_f = np.asarray(Wv, np.float32) * g1f[None, :]
    bv_f = np.asarray(bv, np.float32) + np.asarray(Wv, np.float32) @ b1f
    Wr1_f = np.asarray(Wr1, np.float32) * g2f[None, :]
    br1_f = np.asarray(br1, np.float32) + np.asarray(Wr1, np.float32) @ b2f

    ws_vec = np.asarray(Ws, np.float32)[0, 32:32 + HD]
    Wu = np.einsum("d,rdf->fr", ws_vec, Wv_f.reshape(R, HD, DIM_))
    bu = np.einsum("d,rd->r", ws_vec, bv_f.reshape(R, HD))
    wu8 = np.zeros((DIM_, 9), np.float32)
    wu8[:, 0:8] = Wu * 32.0
    bu_bc = np.zeros((128, 9), np.float32)
    bu_bc[:, 0:8] = bu[None, :]
    bu_bc[:, 8] = 1.0

    # rank-1 mean-correction rows: the device computes xn' = x*rstd and the
    # matmuls subtract mean*rstd via one extra accumulation with these
    # column-sum vectors (row-sums of the folded weights).
    def _pad2(v):
        # [2, n] with a zero second row: lets the rank-1 mean-correction
        # run as a DoubleRow fp8 matmul (0.5 cyc/row)
        m = np.zeros((2, v.shape[0]), np.float32)
        m[0] = v
        return np.ascontiguousarray(m.astype(ml_dtypes.float8_e4m3))
    aug_k = _pad2(Wk_f.sum(axis=1))
    aug_q = _pad2(Wq_f.sum(axis=1))
    aug_v = _pad2(Wv_f.sum(axis=1))
    aug_u = _pad2(wu8.sum(axis=0) / 32.0)
    aug_r1 = _pad2(Wr1_f.sum(axis=1))

    shared = {
        "wu8": np.ascontiguousarray(wu8.astype(ml_dtypes.float8_e4m3)),
        "bu_bc": bu_bc,
        "aug_k": aug_k, "aug_q": aug_q, "aug_v": aug_v, "aug_u": aug_u,
        "aug_r1": aug_r1,
        "wqT": np.ascontiguousarray((Wq_f.T * 32.0
                                     ).astype(ml_dtypes.float8_e4m3)),
        "wkT": np.ascontiguousarray((Wk_f.T * 32.0
                                     ).astype(ml_dtypes.float8_e4m3)),
        "wvT": np.ascontiguousarray((Wv_f.T * 32.0
                                     ).astype(ml_dtypes.float8_e4m3)),
        "wfT": np.ascontiguousarray((np.asarray(Wf, np.float32).T * 32.0
                                     ).astype(ml_dtypes.float8_e4m3)),
        "wr1T": np.ascontiguousarray(
            (Wr1_f.T * 32.0).astype(ml_dtypes.float8_e4m3)),
        "wr2T": np.ascontiguousarray(
            (np.asarray(Wr2, np.float32).T * 32.0).astype(
                ml_dtypes.float8_e4m3)),
        "bk": f32c(bk_f),
        "bq8": f32c(bq_f / 8.0),
        "bf": f32c(bf),
        "br1": f32c(br1_f),
        "br2": f32c(br2),
        "bv_bc": f32c(np.tile(bv_f[None, :], (128, 1))),
        "ones_c": np.ones((128, 128), dtype=np.float32),
        "ident_c": np.eye(128, dtype=np.float32),
        "consts_c": np.tile(np.array([[0.0, EPS]], dtype=np.float32), (128, 1)),
    }

    in_maps = []
    for c in range(8):
        b, half = c // 2, c % 2
        xb = x[b]
        x_rot = np.concatenate(
            [xb[half * Q:(half + 1) * Q], xb[(1 - half) * Q:(2 - half) * Q]], axis=0
        )
        m = dict(shared)
        m["x_fm"] = f32c(x_rot.T)
        in_maps.append(m)

    res = run_bass_kernel_spmd(_get_nc(), in_maps, core_ids=list(range(8)))

    out = np.empty((4, S, D), dtype=np.float32)
    for c in range(8):
        b, half = c // 2, c % 2
        out[b, half * Q:(half + 1) * Q, :] = res.results[c]["out_fm"].T
    return out

